# revision 12
# baseline (speedup 1.0000x reference)
"""Trainium2 Bass kernel for nn_MHSA_37821482008969 (2D rel-pos MHSA).

Strategy: data-parallel over batch (16 batches -> 8 cores x 2). Per (batch,
head) unit, attention is computed fully transposed: S^T = K^T@Q tiles with
y (keys) on partitions, so softmax-normalization sums come from a ones-vector
matmul on PE, the attn matmul needs no transposes of exp(S), and the output
lands directly in the channel-major layout the conv output wants.

Rel-pos biases: built entirely on PE as 64 small shifted matmuls per batch
(32 width shifts x b, 32 height shifts x a) against slices of the rel tables,
writing a [64, 4H*L]-row basis table; the per-(y,x) bias is then folded into
the logits accumulation as one extra K=64 matmul per tile with a constant 0/1
selector lhsT. No DRAM bounce, no DMA gathers, no PE transposes.

All matmul operands are bf16 (fp32 PSUM accumulation); softmax skips the
row-max subtraction (logits are ~N(0,sqrt3), |logit| < 9, exp is safe).
Softmax reciprocal uses the fast approx DVE op (~18 good bits, plenty).
"""
import numpy as np
import ml_dtypes

import concourse.bass as bass
import concourse.mybir as mybir
import concourse.tile as tile
import concourse.bacc as bacc
from concourse.bass_utils import run_bass_kernel_spmd

bf16 = ml_dtypes.bfloat16
FP32 = mybir.dt.float32
BF16 = mybir.dt.bfloat16

HEADS, D, F, DIM = 4, 128, 32, 512
L = F * F           # 1024
B_PER_CORE = 2
N_CORES = 8
AF = mybir.ActivationFunctionType

_cache = {}


def _build():
    nc = bacc.Bacc("TRN2", target_bir_lowering=False, debug=False,
                   num_devices=N_CORES)
    xin = nc.dram_tensor("xin", [B_PER_CORE, 4, 128, L], BF16, kind="ExternalInput").ap()
    wqt = nc.dram_tensor("wqt", [4, 128, DIM], BF16, kind="ExternalInput").ap()
    wkt = nc.dram_tensor("wkt", [4, 128, DIM], BF16, kind="ExternalInput").ap()
    wvt = nc.dram_tensor("wvt", [4, 128, DIM], BF16, kind="ExternalInput").ap()
    relwt = nc.dram_tensor("relwt", [128, 63], BF16, kind="ExternalInput").ap()
    relht = nc.dram_tensor("relht", [128, 63], BF16, kind="ExternalInput").ap()
    sel = nc.dram_tensor("sel", [64, 8 * 128], BF16, kind="ExternalInput").ap()
    ones_col = nc.dram_tensor("ones_col", [128, 1], BF16, kind="ExternalInput").ap()
    ones_row = nc.dram_tensor("ones_row", [1, 128], BF16, kind="ExternalInput").ap()
    out = nc.dram_tensor("out", [B_PER_CORE, DIM, L], FP32, kind="ExternalOutput").ap()

    from contextlib import ExitStack
    ctx = ExitStack()
    with tile.TileContext(nc) as tc, ctx:
        consts = ctx.enter_context(tc.tile_pool(name="consts", bufs=1))
        xpool = ctx.enter_context(tc.tile_pool(name="xpool", bufs=2))
        vtpool = ctx.enter_context(tc.tile_pool(name="vtpool", bufs=2))
        qkpool = ctx.enter_context(tc.tile_pool(name="qkpool", bufs=2))
        biaspool = ctx.enter_context(tc.tile_pool(name="biaspool", bufs=2))
        ptpool = ctx.enter_context(tc.tile_pool(name="ptpool", bufs=2))
        rpool = ctx.enter_context(tc.tile_pool(name="rpool", bufs=3))
        outpool = ctx.enter_context(tc.tile_pool(name="outpool", bufs=2))
        # PSUM budget (8 banks): st ring 2x[128,1024]=4, attn 1x[128,1024]=2,
        # sums 2x[1,512]=2.  V/QK/rel/bc psum tiles share the "st" ring.
        stps = ctx.enter_context(tc.tile_pool(name="stps", bufs=2, space="PSUM"))
        attnps = ctx.enter_context(tc.tile_pool(name="attnps", bufs=1, space="PSUM"))
        sumsps = ctx.enter_context(tc.tile_pool(name="sumsps", bufs=2, space="PSUM"))

        # ---- load constants (issue spread across engine queues so the
        # kernel head isn't serialized on one DMA trigger queue) ----
        _qs = [nc.sync, nc.scalar, nc.gpsimd]
        _qi = [0]

        def cload(ap, shape, tag):
            t = consts.tile(shape, ap.dtype, tag=tag, name=tag)
            _qs[_qi[0] % len(_qs)].dma_start(t[:], ap)
            _qi[0] += 1
            return t
        wq_sb = [cload(wqt[c], [128, DIM], f"wq{c}") for c in range(4)]
        wk_sb = [cload(wkt[c], [128, DIM], f"wk{c}") for c in range(4)]
        wv_sb = [cload(wvt[c], [128, DIM], f"wv{c}") for c in range(4)]
        relw_sb = cload(relwt, [128, 63], "relw")
        relh_sb = cload(relht, [128, 63], "relh")
        sel_sb = cload(sel, [64, 8 * 128], "sel")
        ones_c = cload(ones_col, [128, 1], "onesc")
        ones_r = cload(ones_row, [1, 128], "onesr")

        def load_x(b):
            x_sb = []
            for c in range(4):
                xt = xpool.tile([128, L], BF16, tag=f"x{c}", name=f"x{c}")
                _qs[c % len(_qs)].dma_start(xt[:], xin[b, c])
                x_sb.append(xt)
            return x_sb

        def proj_qk(x_sb):
            # Q, K projections into [d(128), 4h*L] concatenated tiles
            q_cat = qkpool.tile([128, 4 * L], BF16, tag="qcat", name="qcat")
            k_cat = qkpool.tile([128, 4 * L], BF16, tag="kcat", name="kcat")
            for h in range(HEADS):
                for dst, w in ((q_cat, wq_sb), (k_cat, wk_sb)):
                    ps = stps.tile([128, L], FP32, tag="st", name="qkps")
                    for c in range(4):
                        for n in range(2):
                            nc.tensor.matmul(ps[:, n * 512:(n + 1) * 512],
                                             w[c][:, h * 128:(h + 1) * 128],
                                             x_sb[c][:, n * 512:(n + 1) * 512],
                                             start=(c == 0), stop=(c == 3))
                    nc.vector.tensor_copy(dst[:, h * L:(h + 1) * L], ps[:])
            return q_cat, k_cat

        def relbias_chunk(q_cat, bias_all, g):
            # one chunk: 4 width shifts (g<8) or 4 height shifts (g>=8).
            # rel psum lives in the sums ring (idle outside attention);
            # strided rearrange copies go on ACT (ScalarE handles strided
            # PSUM->SBUF at ~620ns vs 2.3us on DVE).
            q4 = q_cat.rearrange("p (h a c) -> p h a c", h=4, a=32, c=32)
            bflat = bias_all.flatten()
            if g < 8:
                # the 4 shift-matmuls write column-interleaved psum (stride 4)
                # so the rearrange copy has 4-elem contiguous runs both sides
                rp = sumsps.tile([32, 512], FP32, tag="sums", name="rp")
                rpf = rp.flatten()
                for j in range(4):
                    bb = 4 * g + j
                    outap = bass.AP(rpf.tensor, rpf.offset + j,
                                    [[512, 32], [4, 128]])
                    nc.tensor.matmul(outap,
                                     relw_sb[:, 31 - bb:63 - bb],
                                     q4[:, :, :, bb], start=True, stop=True)
                # src col(h,a,j) = 4*(32h+a)+j ; dst col = h*1024+32a+4g+j
                srcap = bass.AP(rpf.tensor, rpf.offset,
                                [[512, 32], [128, 4], [4, 32], [1, 4]])
                dstap = bass.AP(bflat.tensor, bflat.offset + 4 * g,
                                [[4 * L, 32], [L, 4], [32, 32], [1, 4]])
                nc.scalar.activation(dstap, srcap, AF.Copy)
            else:
                hp = sumsps.tile([32, 512], FP32, tag="sums", name="hp")
                for j in range(4):
                    aa = 4 * (g - 8) + j
                    nc.tensor.matmul(hp[:, 128 * j:128 * (j + 1)],
                                     relh_sb[:, 31 - aa:63 - aa],
                                     q4[:, :, aa, :], start=True, stop=True)
                # dst col(j,h,c) = h*1024 + 32*(4g+j) + c, partition base 32
                dstap = bass.AP(bflat.tensor,
                                bflat.offset + 32 * 4 * L + 32 * 4 * (g - 8),
                                [[4 * L, 32], [32, 4], [L, 4], [1, 32]])
                nc.scalar.activation(dstap, hp[:], AF.Copy)

        def vchunk(x_sb, vt_sb, yt):
            ps = stps.tile([128, DIM], FP32, tag="st", name="vps")
            for c in range(4):
                nc.tensor.matmul(ps[:], x_sb[c][:, yt * 128:(yt + 1) * 128],
                                 wv_sb[c][:], start=(c == 0), stop=(c == 3))
            vt = vtpool.tile([128, DIM], BF16, tag=f"vt{yt}", name=f"vt{yt}")
            nc.vector.tensor_copy(vt[:], ps[:])
            vt_sb[yt] = vt

        def relbias_and_v(x_sb, q_cat):
            # rel-pos bias basis table bias_all [64, 4h*L]
            #   rows 0:32  = width rows  r (selected by y%32)
            #   rows 32:64 = height rows s (selected by y//32)
            # interleaved with the V^T projection so PE work covers the
            # ACT-bound rearrange copies.
            bias_all = biaspool.tile([64, 4 * L], BF16, tag="bias", name="bias")
            vt_sb = [None] * 8
            for g in range(16):
                relbias_chunk(q_cat, bias_all, g)
                if g % 2 == 0:
                    vchunk(x_sb, vt_sb, g // 2)
            return bias_all, vt_sb

        def attention(b, h, q_cat, k_cat, bias_all, vt_sb):
            hq = q_cat[:, h * L:(h + 1) * L]
            pt_sb = []
            for yt in range(8):
                st = stps.tile([128, L], FP32, tag="st", name="st")
                klhs = k_cat[:, h * L + yt * 128:h * L + (yt + 1) * 128]
                for n in range(2):
                    nc.tensor.matmul(st[:, n * 512:(n + 1) * 512], klhs,
                                     hq[:, n * 512:(n + 1) * 512],
                                     start=True, stop=False)
                for n in range(2):
                    nc.tensor.matmul(st[:, n * 512:(n + 1) * 512],
                                     sel_sb[:, yt * 128:(yt + 1) * 128],
                                     bias_all[:, h * L + n * 512:h * L + (n + 1) * 512],
                                     start=False, stop=True)
                pt = ptpool.tile([128, L], BF16, tag=f"pt{yt}", name=f"pt{yt}")
                nc.scalar.activation(pt[:], st[:], AF.Exp)
                pt_sb.append(pt)

            # softmax denominators: pairwise add-tree over the 8 pt tiles on
            # GpSimd (idle) + DVE, then a single ones-matmul per half.  Cuts
            # 14 of 16 f=512 sums-matmuls per head off the PE.  bf16 tree
            # rounding adds ~0.1% to the denominator — well inside tolerance.
            s2 = []
            for i in range(4):
                t = ptpool.tile([128, L], BF16, tag=f"s2_{i}", name=f"s2_{i}")
                nc.gpsimd.tensor_add(t[:], pt_sb[2 * i][:], pt_sb[2 * i + 1][:])
                s2.append(t)
            s4 = []
            for i in range(2):
                t = ptpool.tile([128, L], BF16, tag=f"s4_{i}", name=f"s4_{i}")
                nc.vector.tensor_add(t[:], s2[2 * i][:], s2[2 * i + 1][:])
                s4.append(t)
            s8 = ptpool.tile([128, L], BF16, tag="s8", name="s8")
            nc.vector.tensor_add(s8[:], s4[0][:], s4[1][:])
            sums = [sumsps.tile([1, 512], FP32, tag="sums", name="sums")
                    for _ in range(2)]
            for n in range(2):
                nc.tensor.matmul(sums[n][:], ones_c[:],
                                 s8[:, n * 512:(n + 1) * 512],
                                 start=True, stop=True)
            recipb = rpool.tile([1, L], BF16, tag="recipb", name="recipb")
            for n in range(2):
                recip = rpool.tile([1, 512], FP32, tag=f"recip{n}", name=f"recip{n}")
                nc.vector.reciprocal_approx_fast(recip[:], sums[n][:])
                nc.vector.tensor_copy(recipb[:, n * 512:(n + 1) * 512],
                                      recip[:])

            attn = attnps.tile([128, L], FP32, tag="attn", name="attn")
            for yt in range(8):
                vlhs = vt_sb[yt][:, h * 128:(h + 1) * 128]
                for n in range(2):
                    nc.tensor.matmul(attn[:, n * 512:(n + 1) * 512], vlhs,
                                     pt_sb[yt][:, n * 512:(n + 1) * 512],
                                     start=(yt == 0), stop=(yt == 7))

            bc = stps.tile([128, L], FP32, tag="st", name="bc")
            for n in range(2):
                nc.tensor.matmul(bc[:, n * 512:(n + 1) * 512], ones_r[:],
                                 recipb[:, n * 512:(n + 1) * 512],
                                 start=True, stop=True)
            bc_sb = outpool.tile([128, L], FP32, tag="bcsb", name="bcsb")
            nc.vector.tensor_copy(bc_sb[:], bc[:])
            o_sb = outpool.tile([128, L], FP32, tag="osb", name="osb")
            nc.vector.tensor_mul(o_sb[:], attn[:], bc_sb[:])
            nc.sync.dma_start(out[b, h * 128:(h + 1) * 128, :], o_sb[:])

        # Software pipeline: emit b1's projection/rel-bias phases in small
        # chunks interleaved into b0's attention heads so the PE never drains
        # (keeps the HAM clock gate at full rate), the ACT/DVE copy chains
        # overlap PE, and the shared sums-ring never stalls a head's sums.
        x0 = load_x(0)
        q0, k0 = proj_qk(x0)
        bias0, vt0 = relbias_and_v(x0, q0)
        attention(0, 0, q0, k0, bias0, vt0)
        x1 = load_x(1)
        q1, k1 = proj_qk(x1)
        attention(0, 1, q0, k0, bias0, vt0)
        bias1 = biaspool.tile([64, 4 * L], BF16, tag="bias", name="bias")
        vt1 = [None] * 8
        for g in range(5):
            relbias_chunk(q1, bias1, g)
            if g % 2 == 0:
                vchunk(x1, vt1, g // 2)
        attention(0, 2, q0, k0, bias0, vt0)
        for g in range(5, 11):
            relbias_chunk(q1, bias1, g)
            if g % 2 == 0:
                vchunk(x1, vt1, g // 2)
        attention(0, 3, q0, k0, bias0, vt0)
        for g in range(11, 16):
            relbias_chunk(q1, bias1, g)
            if g % 2 == 0:
                vchunk(x1, vt1, g // 2)
        for h in range(HEADS):
            attention(1, h, q1, k1, bias1, vt1)

    nc.compile()
    return nc


def _prep_inputs(featuremap, w_qk, w_v, rel_height, rel_width):
    scale = D ** -0.5
    wqt = np.ascontiguousarray(w_qk[:DIM].T * scale).astype(bf16).reshape(4, 128, DIM)
    wkt = np.ascontiguousarray(w_qk[DIM:].T).astype(bf16).reshape(4, 128, DIM)
    wvt = np.ascontiguousarray(w_v.T).astype(bf16).reshape(4, 128, DIM)
    relwt = np.ascontiguousarray(rel_width.T).astype(bf16)
    relht = np.ascontiguousarray(rel_height.T).astype(bf16)
    yy = np.arange(128)
    sel = np.zeros((64, 8 * 128), np.float32)
    for yt in range(8):
        sel[yy % 32, yt * 128 + yy] = 1.0
        sel[32 + yt * 4 + yy // 32, yt * 128 + yy] = 1.0
    sel = sel.astype(bf16)
    ones_col = np.ones((128, 1), bf16)
    ones_row = np.ones((1, 128), bf16)
    common = dict(wqt=wqt, wkt=wkt, wvt=wvt, relwt=relwt, relht=relht,
                  sel=sel, ones_col=ones_col, ones_row=ones_row)
    xin = featuremap.reshape(16, DIM, L).astype(bf16).reshape(
        N_CORES, B_PER_CORE, 4, 128, L)
    return [dict(common, xin=np.ascontiguousarray(xin[i])) for i in range(N_CORES)]


def kernel(featuremap, w_qk, w_v, rel_height, rel_width, _trace=False, _tmpdir=None):
    if "nc" not in _cache:
        _cache["nc"] = _build()
    nc = _cache["nc"]
    in_maps = _prep_inputs(featuremap, w_qk, w_v, rel_height, rel_width)
    res = run_bass_kernel_spmd(nc, in_maps, list(range(N_CORES)),
                               trace=_trace, tmpdir=_tmpdir)
    _cache["last_result"] = res
    full = np.concatenate([res.results[i]["out"] for i in range(N_CORES)], axis=0)
    return full.reshape(16, DIM, F, F)


# revision 15
# speedup vs baseline: 1.1494x; 1.1494x over previous
"""Trainium2 Bass kernel for nn_MHSA_37821482008969 (2D rel-pos MHSA).

Strategy: data-parallel over batch (16 batches -> 8 cores x 2). Per (batch,
head) unit, attention is computed fully transposed: S^T = K^T@Q tiles with
y (keys) on partitions, so softmax-normalization sums come from a ones-vector
matmul on PE, the attn matmul needs no transposes of exp(S), and the output
lands directly in the channel-major layout the conv output wants.

Rel-pos biases: built entirely on PE as 64 small shifted matmuls per batch
(32 width shifts x b, 32 height shifts x a) against slices of the rel tables,
writing a [64, 4H*L]-row basis table; the per-(y,x) bias is then folded into
the logits accumulation as one extra K=64 matmul per tile with a constant 0/1
selector lhsT. No DRAM bounce, no DMA gathers, no PE transposes.

All matmul operands are bf16 (fp32 PSUM accumulation); softmax skips the
row-max subtraction (logits are ~N(0,sqrt3), |logit| < 9, exp is safe).
Softmax reciprocal uses the fast approx DVE op (~18 good bits, plenty).
"""
import numpy as np
import ml_dtypes

import concourse.bass as bass
import concourse.mybir as mybir
import concourse.tile as tile
import concourse.bacc as bacc
from concourse.bass_utils import run_bass_kernel_spmd

bf16 = ml_dtypes.bfloat16
FP32 = mybir.dt.float32
BF16 = mybir.dt.bfloat16

HEADS, D, F, DIM = 4, 128, 32, 512
L = F * F           # 1024
B_PER_CORE = 2
N_CORES = 8
AF = mybir.ActivationFunctionType

_cache = {}


def _build():
    nc = bacc.Bacc("TRN2", target_bir_lowering=False, debug=False,
                   num_devices=N_CORES)
    xin = nc.dram_tensor("xin", [B_PER_CORE, 4, 128, L], BF16, kind="ExternalInput").ap()
    wqt = nc.dram_tensor("wqt", [4, 128, DIM], BF16, kind="ExternalInput").ap()
    wkt = nc.dram_tensor("wkt", [4, 128, DIM], BF16, kind="ExternalInput").ap()
    wvt = nc.dram_tensor("wvt", [4, 128, DIM], BF16, kind="ExternalInput").ap()
    relwt = nc.dram_tensor("relwt", [128, 63], BF16, kind="ExternalInput").ap()
    relht = nc.dram_tensor("relht", [128, 63], BF16, kind="ExternalInput").ap()
    sel = nc.dram_tensor("sel", [64, 8 * 128], BF16, kind="ExternalInput").ap()
    ones_col = nc.dram_tensor("ones_col", [128, 1], BF16, kind="ExternalInput").ap()
    ones_row = nc.dram_tensor("ones_row", [1, 128], BF16, kind="ExternalInput").ap()
    out = nc.dram_tensor("out", [B_PER_CORE, DIM, L], FP32, kind="ExternalOutput").ap()

    from contextlib import ExitStack
    ctx = ExitStack()
    with tile.TileContext(nc) as tc, ctx:
        consts = ctx.enter_context(tc.tile_pool(name="consts", bufs=1))
        xpool = ctx.enter_context(tc.tile_pool(name="xpool", bufs=2))
        vtpool = ctx.enter_context(tc.tile_pool(name="vtpool", bufs=2))
        qkpool = ctx.enter_context(tc.tile_pool(name="qkpool", bufs=2))
        biaspool = ctx.enter_context(tc.tile_pool(name="biaspool", bufs=2))
        ptpool = ctx.enter_context(tc.tile_pool(name="ptpool", bufs=2))
        rpool = ctx.enter_context(tc.tile_pool(name="rpool", bufs=3))
        outpool = ctx.enter_context(tc.tile_pool(name="outpool", bufs=2))
        # PSUM budget (8 banks): st ring 2x[128,1024]=4, attn 1x[128,1024]=2,
        # sums 2x[1,512]=2.  V/QK/rel/bc psum tiles share the "st" ring.
        stps = ctx.enter_context(tc.tile_pool(name="stps", bufs=2, space="PSUM"))
        attnps = ctx.enter_context(tc.tile_pool(name="attnps", bufs=1, space="PSUM"))
        sumsps = ctx.enter_context(tc.tile_pool(name="sumsps", bufs=2, space="PSUM"))

        # ---- load constants (issue spread across engine queues so the
        # kernel head isn't serialized on one DMA trigger queue) ----
        _qs = [nc.sync, nc.scalar, nc.gpsimd]
        _qi = [0]

        def cload(ap, shape, tag):
            t = consts.tile(shape, ap.dtype, tag=tag, name=tag)
            _qs[_qi[0] % len(_qs)].dma_start(t[:], ap)
            _qi[0] += 1
            return t
        def load_x(b):
            x_sb = []
            for c in range(4):
                xt = xpool.tile([128, L], BF16, tag=f"x{c}", name=f"x{c}")
                _qs[c % len(_qs)].dma_start(xt[:], xin[b, c])
                x_sb.append(xt)
            return x_sb

        # order matters for the kernel head: the first QK matmuls need wq + x,
        # so issue those DMAs before the rest of the constants.
        wq_sb = [cload(wqt[c], [128, DIM], f"wq{c}") for c in range(4)]
        x0_pre = load_x(0)
        wk_sb = [cload(wkt[c], [128, DIM], f"wk{c}") for c in range(4)]
        wv_sb = [cload(wvt[c], [128, DIM], f"wv{c}") for c in range(4)]
        relw_sb = cload(relwt, [128, 63], "relw")
        relh_sb = cload(relht, [128, 63], "relh")
        sel_sb = cload(sel, [64, 8 * 128], "sel")
        ones_c = cload(ones_col, [128, 1], "onesc")
        ones_r = cload(ones_row, [1, 128], "onesr")

        def proj_qk(x_sb):
            # Q, K projections into [d(128), 4h*L] concatenated tiles
            q_cat = qkpool.tile([128, 4 * L], BF16, tag="qcat", name="qcat")
            k_cat = qkpool.tile([128, 4 * L], BF16, tag="kcat", name="kcat")
            for h in range(HEADS):
                for dst, w in ((q_cat, wq_sb), (k_cat, wk_sb)):
                    ps = stps.tile([128, L], FP32, tag="st", name="qkps")
                    for c in range(4):
                        for n in range(2):
                            nc.tensor.matmul(ps[:, n * 512:(n + 1) * 512],
                                             w[c][:, h * 128:(h + 1) * 128],
                                             x_sb[c][:, n * 512:(n + 1) * 512],
                                             start=(c == 0), stop=(c == 3))
                    nc.vector.tensor_copy(dst[:, h * L:(h + 1) * L], ps[:])
            return q_cat, k_cat

        def relbias_chunk(q_cat, bias_all, g):
            # one chunk: 4 width shifts (g<8) or 4 height shifts (g>=8).
            # rel psum lives in the sums ring (idle outside attention);
            # strided rearrange copies go on ACT (ScalarE handles strided
            # PSUM->SBUF at ~620ns vs 2.3us on DVE).
            q4 = q_cat.rearrange("p (h a c) -> p h a c", h=4, a=32, c=32)
            bflat = bias_all.flatten()
            if g < 8:
                # the 4 shift-matmuls write column-interleaved psum (stride 4)
                # so the rearrange copy has 4-elem contiguous runs both sides
                rp = sumsps.tile([32, 512], FP32, tag="sums", name="rp")
                rpf = rp.flatten()
                for j in range(4):
                    bb = 4 * g + j
                    outap = bass.AP(rpf.tensor, rpf.offset + j,
                                    [[512, 32], [4, 128]])
                    nc.tensor.matmul(outap,
                                     relw_sb[:, 31 - bb:63 - bb],
                                     q4[:, :, :, bb], start=True, stop=True)
                # src col(h,a,j) = 4*(32h+a)+j ; dst col = h*1024+32a+4g+j
                srcap = bass.AP(rpf.tensor, rpf.offset,
                                [[512, 32], [128, 4], [4, 32], [1, 4]])
                dstap = bass.AP(bflat.tensor, bflat.offset + 4 * g,
                                [[4 * L, 32], [L, 4], [32, 32], [1, 4]])
                nc.scalar.activation(dstap, srcap, AF.Copy)
            else:
                hp = sumsps.tile([32, 512], FP32, tag="sums", name="hp")
                for j in range(4):
                    aa = 4 * (g - 8) + j
                    nc.tensor.matmul(hp[:, 128 * j:128 * (j + 1)],
                                     relh_sb[:, 31 - aa:63 - aa],
                                     q4[:, :, aa, :], start=True, stop=True)
                # dst col(j,h,c) = h*1024 + 32*(4g+j) + c, partition base 32
                dstap = bass.AP(bflat.tensor,
                                bflat.offset + 32 * 4 * L + 32 * 4 * (g - 8),
                                [[4 * L, 32], [32, 4], [L, 4], [1, 32]])
                nc.scalar.activation(dstap, hp[:], AF.Copy)

        def vchunk(x_sb, vt_sb, yt):
            ps = stps.tile([128, DIM], FP32, tag="st", name="vps")
            for c in range(4):
                nc.tensor.matmul(ps[:], x_sb[c][:, yt * 128:(yt + 1) * 128],
                                 wv_sb[c][:], start=(c == 0), stop=(c == 3))
            vt = vtpool.tile([128, DIM], BF16, tag=f"vt{yt}", name=f"vt{yt}")
            nc.vector.tensor_copy(vt[:], ps[:])
            vt_sb[yt] = vt

        def relbias_and_v(x_sb, q_cat):
            # rel-pos bias basis table bias_all [64, 4h*L]
            #   rows 0:32  = width rows  r (selected by y%32)
            #   rows 32:64 = height rows s (selected by y//32)
            # interleaved with the V^T projection so PE work covers the
            # ACT-bound rearrange copies.
            bias_all = biaspool.tile([64, 4 * L], BF16, tag="bias", name="bias")
            vt_sb = [None] * 8
            for g in range(16):
                relbias_chunk(q_cat, bias_all, g)
                if g % 2 == 0:
                    vchunk(x_sb, vt_sb, g // 2)
            return bias_all, vt_sb

        def attention(b, h, q_cat, k_cat, bias_all, vt_sb):
            hq = q_cat[:, h * L:(h + 1) * L]
            pt_sb = []
            for yt in range(8):
                st = stps.tile([128, L], FP32, tag="st", name="st")
                klhs = k_cat[:, h * L + yt * 128:h * L + (yt + 1) * 128]
                for n in range(2):
                    nc.tensor.matmul(st[:, n * 512:(n + 1) * 512], klhs,
                                     hq[:, n * 512:(n + 1) * 512],
                                     start=True, stop=False)
                for n in range(2):
                    nc.tensor.matmul(st[:, n * 512:(n + 1) * 512],
                                     sel_sb[:, yt * 128:(yt + 1) * 128],
                                     bias_all[:, h * L + n * 512:h * L + (n + 1) * 512],
                                     start=False, stop=True)
                pt = ptpool.tile([128, L], BF16, tag=f"pt{yt}", name=f"pt{yt}")
                nc.scalar.activation(pt[:], st[:], AF.Exp)
                pt_sb.append(pt)

            sums = [sumsps.tile([1, 512], FP32, tag="sums", name="sums")
                    for _ in range(2)]
            for yt in range(8):
                for n in range(2):
                    nc.tensor.matmul(sums[n][:], ones_c[:],
                                     pt_sb[yt][:, n * 512:(n + 1) * 512],
                                     start=(yt == 0), stop=(yt == 7))
            recipb = rpool.tile([1, L], BF16, tag="recipb", name="recipb")
            for n in range(2):
                recip = rpool.tile([1, 512], FP32, tag=f"recip{n}", name=f"recip{n}")
                nc.vector.reciprocal_approx_fast(recip[:], sums[n][:])
                nc.vector.tensor_copy(recipb[:, n * 512:(n + 1) * 512],
                                      recip[:])

            attn = attnps.tile([128, L], FP32, tag="attn", name="attn")
            for yt in range(8):
                vlhs = vt_sb[yt][:, h * 128:(h + 1) * 128]
                for n in range(2):
                    nc.tensor.matmul(attn[:, n * 512:(n + 1) * 512], vlhs,
                                     pt_sb[yt][:, n * 512:(n + 1) * 512],
                                     start=(yt == 0), stop=(yt == 7))

            bc = stps.tile([128, L], FP32, tag="st", name="bc")
            for n in range(2):
                nc.tensor.matmul(bc[:, n * 512:(n + 1) * 512], ones_r[:],
                                 recipb[:, n * 512:(n + 1) * 512],
                                 start=True, stop=True)
            bc_sb = outpool.tile([128, L], FP32, tag="bcsb", name="bcsb")
            nc.vector.tensor_copy(bc_sb[:], bc[:])
            o_sb = outpool.tile([128, L], FP32, tag="osb", name="osb")
            nc.vector.tensor_mul(o_sb[:], attn[:], bc_sb[:])
            nc.sync.dma_start(out[b, h * 128:(h + 1) * 128, :], o_sb[:])

        # Software pipeline: emit b1's projection/rel-bias phases in small
        # chunks interleaved into b0's attention heads so the PE never drains
        # (keeps the HAM clock gate at full rate), the ACT/DVE copy chains
        # overlap PE, and the shared sums-ring never stalls a head's sums.
        x0 = x0_pre
        q0, k0 = proj_qk(x0)
        bias0, vt0 = relbias_and_v(x0, q0)
        attention(0, 0, q0, k0, bias0, vt0)
        x1 = load_x(1)
        q1, k1 = proj_qk(x1)
        attention(0, 1, q0, k0, bias0, vt0)
        bias1 = biaspool.tile([64, 4 * L], BF16, tag="bias", name="bias")
        vt1 = [None] * 8
        for g in range(5):
            relbias_chunk(q1, bias1, g)
            if g % 2 == 0:
                vchunk(x1, vt1, g // 2)
        attention(0, 2, q0, k0, bias0, vt0)
        for g in range(5, 11):
            relbias_chunk(q1, bias1, g)
            if g % 2 == 0:
                vchunk(x1, vt1, g // 2)
        attention(0, 3, q0, k0, bias0, vt0)
        for g in range(11, 16):
            relbias_chunk(q1, bias1, g)
            if g % 2 == 0:
                vchunk(x1, vt1, g // 2)
        for h in range(HEADS):
            attention(1, h, q1, k1, bias1, vt1)

    nc.compile()
    return nc


def _prep_inputs(featuremap, w_qk, w_v, rel_height, rel_width):
    scale = D ** -0.5
    wqt = np.ascontiguousarray(w_qk[:DIM].T * scale).astype(bf16).reshape(4, 128, DIM)
    wkt = np.ascontiguousarray(w_qk[DIM:].T).astype(bf16).reshape(4, 128, DIM)
    wvt = np.ascontiguousarray(w_v.T).astype(bf16).reshape(4, 128, DIM)
    relwt = np.ascontiguousarray(rel_width.T).astype(bf16)
    relht = np.ascontiguousarray(rel_height.T).astype(bf16)
    yy = np.arange(128)
    sel = np.zeros((64, 8 * 128), np.float32)
    for yt in range(8):
        sel[yy % 32, yt * 128 + yy] = 1.0
        sel[32 + yt * 4 + yy // 32, yt * 128 + yy] = 1.0
    sel = sel.astype(bf16)
    ones_col = np.ones((128, 1), bf16)
    ones_row = np.ones((1, 128), bf16)
    common = dict(wqt=wqt, wkt=wkt, wvt=wvt, relwt=relwt, relht=relht,
                  sel=sel, ones_col=ones_col, ones_row=ones_row)
    xin = featuremap.reshape(16, DIM, L).astype(bf16).reshape(
        N_CORES, B_PER_CORE, 4, 128, L)
    return [dict(common, xin=np.ascontiguousarray(xin[i])) for i in range(N_CORES)]


def kernel(featuremap, w_qk, w_v, rel_height, rel_width, _trace=False, _tmpdir=None):
    if "nc" not in _cache:
        _cache["nc"] = _build()
    nc = _cache["nc"]
    in_maps = _prep_inputs(featuremap, w_qk, w_v, rel_height, rel_width)
    res = run_bass_kernel_spmd(nc, in_maps, list(range(N_CORES)),
                               trace=_trace, tmpdir=_tmpdir)
    _cache["last_result"] = res
    full = np.concatenate([res.results[i]["out"] for i in range(N_CORES)], axis=0)
    return full.reshape(16, DIM, F, F)
